# revision 2
# baseline (speedup 1.0000x reference)
"""Trainium2 Bass kernel for AssociativeMemoryModule (causal linear attention).

Sharding (2D, batch x head-pair): core c -> batch b=c//4, heads {2q, 2q+1}
with q=c%4. Output sharding: core c owns token slice [128c, 128c+128) of
BOTH batches, so a single 8-core AllToAll moves exactly one 32KB chunk per
(src, dest) pair with zero padding.

Per core:
  1. load xT for its batch (bf16, host pre-transposed) + packed weights;
     a tiny warm-up AllToAll fires first so the real one skips the
     first-collective setup latency,
  2. project q,k feature-major ([q0;q1]/[k0;k1] rows; phi=elu+1 =
     min(exp,1)+relu: ACT exp+relu, DVE min+add), v token-major directly
     (bias via K=1 matmul),
  3. k -> token-major via PE transpose (both heads per chunk in one op),
  4. chunked causal linear attention (C=128): kv outer products col-tiled
     (both heads concurrent, one PSUM tile), DVE prefix chain, po = intra
     + inter in PSUM, denominators via ones-column, scaled bf16 output
     token-major,
  5. single 8-core AllToAll (256 KB) redistributes to token-slices,
  6. one gather DMA, PE transposes, o-projection for [512, 256] out.T.
Host reassembles 8 (512, 256) o.T slices.
"""
import sys

import numpy as np

sys.path.insert(0, "/opt/trn_rl_repo")

H, HD, D = 8, 64, 512
B, T = 2, 1024
C = 128               # attention chunk
NCH = T // C          # 8 chunks per core (its batch)
NF = D // 128         # 4 feature tiles
TS = 256              # per-core output token columns
CBW = NF * 896 + 128  # packed bf16 consts width

_CACHE = {}


def _build():
    if "nc" in _CACHE:
        return _CACHE["nc"]
    import concourse.mybir as mybir
    import concourse.tile as tile
    from concourse import bacc
    from concourse.bass import ts

    import ml_dtypes

    f32 = mybir.dt.float32
    bf16 = mybir.dt.bfloat16
    AF = mybir.ActivationFunctionType
    ALU = mybir.AluOpType

    nc = bacc.Bacc("TRN2", target_bir_lowering=False, debug=False, num_devices=8,
                   num_swdge_queues=4)

    iden_np = np.eye(C, dtype=ml_dtypes.bfloat16)
    # packed params (host-side layout, see _in_maps):
    #   cbf: [128, NF*(256 wqk + 128 wv + 512 wo) + 128 bv_row] bf16
    #   cf32: [128, 2 bqk + 1 pad + 4 bo + 256 mask2] f32
    xT = nc.declare_dram_parameter("xT", [D, T], bf16, isOutput=False)
    cbf = nc.declare_dram_parameter("cbf", [128, CBW], bf16, isOutput=False)
    cf32 = nc.declare_dram_parameter("cf32", [128, 263], f32, isOutput=False)
    out = nc.declare_dram_parameter("out", [D, TS], f32, isOutput=True)
    iden_d = nc.inline_tensor(iden_np, "iden128")

    with tile.TileContext(nc) as tc:
        with (
            tc.tile_pool(name="consts", bufs=1) as consts,
            tc.tile_pool(name="dram", bufs=1, space="DRAM") as dram,
        ):
            # ---- resident SBUF tensors ----
            xt_sb = consts.tile([128, NF, T], bf16)        # x.T f-tiles
            wqk_sb = consts.tile([128, NF, 256], bf16)     # cols: q0q1(128)|k0k1(128)
            wv_sb = consts.tile([128, NF, 128], bf16)      # v0|v1
            wo_sb = consts.tile([128, NF, D], bf16)        # ki-tile = head-pair src
            bqk_sb = consts.tile([128, 2], f32)
            bo_sb = consts.tile([128, NF], f32)
            mask2_sb = consts.tile([C, 2 * C], f32)        # [mask|mask]
            bv_col = consts.tile([128, 1], f32)            # [bv0;bv1]
            iden_sb = consts.tile([C, C], bf16)            # PE transpose identity

            q_fm = consts.tile([128, T], bf16)             # rows 0-63 q0, 64-127 q1
            k_fm = consts.tile([128, T], bf16)
            v_fm = consts.tile([128, T], bf16)             # rows v0, v1
            k_tok = consts.tile([128, NCH, 128], bf16)     # token-major k (2 heads)
            v_sb = consts.tile([128, NCH, 130], bf16)      # [1|v0|1|v1] per chunk
            sm = consts.tile([128, NCH, 2, C], bf16)       # masked scores per head
            Sf = consts.tile([128, NCH - 1, 65], f32)      # running state (f32)
            Sb = consts.tile([128, NCH - 1, 65], bf16)     # bf16 copy for PE
            ao = consts.tile([128, NCH, 128], bf16)        # attn out token-major
            gr_sb = consts.tile([128, 8, C], bf16)         # gathered (token-major)
            g_sb = consts.tile([128, 8, C], bf16)          # feature-major

            # A2A bounce buffers: shard d = my heads' outputs for chunk d
            cc_in = dram.tile([8, C, 128], bf16, name="cc_in")
            cc_out = dram.tile([8, C, 128], bf16, name="cc_out")
            ccw_in = dram.tile([8, 1024], bf16, name="ccw_in")
            ccw_out = dram.tile([8, 1024], bf16, name="ccw_out")

            nc.vector.memset(v_sb[:, :, 0:1], 1.0)
            nc.vector.memset(v_sb[:, :, 65:66], 1.0)

            # ---- input DMAs ----
            # gpsimd SWDGE queues carry xT (critical for first matmul);
            # sync HWDGE carries the small consts; scalar issues nothing
            # so the first phi ACT is not delayed.
            cbf_r = cbf.ap()[:, 0:NF * 896].rearrange("p (f c) -> p f c", f=NF)
            xre = xT.ap().rearrange("(f p) t -> p f t", p=128)
            nc.gpsimd.dma_start(xt_sb[:, 0:2, 0:512], xre[:, 0:2, 0:512])
            nc.gpsimd.dma_start(xt_sb[:, 2:4, 0:512], xre[:, 2:4, 0:512])
            nc.gpsimd.dma_start(xt_sb[:, 0:2, 512:1024], xre[:, 0:2, 512:1024])
            nc.gpsimd.dma_start(xt_sb[:, 2:4, 512:1024], xre[:, 2:4, 512:1024])
            nc.sync.dma_start(wqk_sb[:, :, :], cbf_r[:, :, 0:256])
            nc.sync.dma_start(bqk_sb[:, :], cf32[:, 0:2])
            nc.sync.dma_start(bv_col[:, :], cf32[:, 2:3])
            nc.sync.dma_start(wv_sb[:, :, :], cbf_r[:, :, 256:384])
            nc.sync.dma_start(iden_sb[:, :], iden_d[:, :])
            nc.sync.dma_start(mask2_sb[:, :], cf32[:, 7:263])
            nc.sync.dma_start(bo_sb[:, :], cf32[:, 3:7])

            # warm-up collective: absorbs rank rendezvous + first-collective
            # ncfw setup while compute runs
            WARMUP_CC = False
            if WARMUP_CC:
                ccw_src = consts.tile([8, 1024], bf16)
                nc.vector.memset(ccw_src[:, :], 0.0)
                nc.gpsimd.dma_start(ccw_in[:, :], ccw_src[:, :])
                nc.gpsimd.collective_compute(
                    "AllToAll", mybir.AluOpType.bypass,
                    replica_groups=[list(range(8))],
                    ins=[ccw_in.opt()], outs=[ccw_out.opt()],
                )

            with (
                tc.tile_pool(name="psA", bufs=3, space="PSUM") as psA,
                tc.tile_pool(name="tmp", bufs=4) as tmp,
                tc.tile_pool(name="drp", bufs=2) as drp,
                tc.tile_pool(name="fin", bufs=2) as fin,
            ):
                def proj_mm(tt, psP):
                    sl = ts(tt, 512)
                    paQ = psP.tile([128, 512], f32, tag="pa", name=f"paQ{tt}")
                    for f in range(NF):
                        nc.tensor.matmul(paQ, wqk_sb[:, f, 0:128], xt_sb[:, f, sl],
                                         start=(f == 0), stop=(f == NF - 1))
                    paK = psP.tile([128, 512], f32, tag="pa", name=f"paK{tt}")
                    for f in range(NF):
                        nc.tensor.matmul(paK, wqk_sb[:, f, 128:256], xt_sb[:, f, sl],
                                         start=(f == 0), stop=(f == NF - 1))
                    paV = psP.tile([128, 512], f32, tag="pa", name=f"paV{tt}")
                    for f in range(NF):
                        nc.tensor.matmul(paV, wv_sb[:, f, :], xt_sb[:, f, sl],
                                         start=(f == 0), stop=(f == NF - 1))
                    nc.scalar.activation(v_fm[:, sl], paV, AF.Identity,
                                         bias=bv_col[:, 0:1])
                    # phi = min(exp(z),1) + relu(z), z = pa + bias
                    for pa, dst, bcol in ((paQ, q_fm, 0), (paK, k_fm, 1)):
                        ee = tmp.tile([128, 512], f32, tag="ee", name=f"ee{tt}{bcol}")
                        rr = tmp.tile([128, 512], f32, tag="rr", name=f"rr{tt}{bcol}")
                        nc.scalar.activation(ee, pa, AF.Exp,
                                             bias=bqk_sb[:, bcol:bcol + 1])
                        nc.scalar.activation(rr, pa, AF.Relu,
                                             bias=bqk_sb[:, bcol:bcol + 1])
                        nc.vector.tensor_scalar_min(ee, ee, 1.0)
                        nc.vector.tensor_tensor(dst[:, sl], ee, rr, ALU.add)

                def proj_rest(tt, psX):
                    # v, k -> token-major via PE transpose (both heads at once)
                    for jj in range(4):
                        ch = tt * 4 + jj
                        pw = psX.tile([128, 256], bf16, tag="x", name=f"pw{ch}")
                        nc.tensor.transpose(pw[:, 0:128], v_fm[:, ts(ch, C)],
                                            iden_sb[:])
                        nc.scalar.activation(
                            v_sb[:, ch, :].rearrange("p (two c) -> p two c",
                                                     two=2)[:, :, 1:65],
                            pw[:, 0:128].rearrange("p (two c) -> p two c", two=2),
                            AF.Copy)
                    for jj in range(4):
                        ch = tt * 4 + jj
                        pt = psX.tile([128, 256], bf16, tag="x", name=f"pt{ch}")
                        nc.tensor.transpose(pt[:, 0:128], k_fm[:, ts(ch, C)],
                                            iden_sb[:])
                        if jj % 2 == 0:
                            nc.vector.tensor_copy(k_tok[:, ch, :], pt[:, 0:128])
                        else:
                            nc.scalar.copy(k_tok[:, ch, :], pt[:, 0:128])
                    # masked scores per head
                    for jj in range(4):
                        ch = tt * 4 + jj
                        cs = ts(ch, C)
                        for h in range(2):
                            sc = psX.tile([128, 256], f32, tag="x",
                                          name=f"sc{ch}{h}")
                            nc.tensor.matmul(sc[:, 0:128],
                                             k_fm[64 * h:64 * h + 64, cs],
                                             q_fm[64 * h:64 * h + 64, cs],
                                             start=True, stop=True)
                            nc.vector.tensor_tensor(sm[:, ch, h, :], sc[:, 0:128],
                                                    mask2_sb[:, 0:128], ALU.mult)

                def kv_chunks():
                    # kv outer products (both heads col-tiled into one tile);
                    # DVE prefix chain, off-chain bf16 converts on scalar
                    for ch in range(NCH - 1):
                        pkv = psA.tile([128, 130], f32, tag="a", name=f"kv{ch}")
                        nc.tensor.matmul(pkv[0:64, 0:65], k_tok[:, ch, 0:64],
                                         v_sb[:, ch, 0:65], start=True, stop=True,
                                         tile_position=(0, 0))
                        nc.tensor.matmul(pkv[64:128, 0:65], k_tok[:, ch, 64:128],
                                         v_sb[:, ch, 65:130], start=True, stop=True,
                                         tile_position=(0, 64))
                        if ch == 0:
                            nc.vector.tensor_copy(Sf[:, 0, :], pkv[:, 0:65])
                        else:
                            nc.vector.tensor_tensor(Sf[:, ch, :], Sf[:, ch - 1, :],
                                                    pkv[:, 0:65], ALU.add)
                        nc.scalar.copy(Sb[:, ch, :], Sf[:, ch, :])

                def po_chunks():
                    for ch in range(NCH):
                        po = psA.tile([128, 130], f32, tag="a", name=f"po{ch}")
                        only = (ch == 0)
                        for h in range(2):
                            nc.tensor.matmul(po[:, 65 * h:65 * h + 65],
                                             sm[:, ch, h, :],
                                             v_sb[:, ch, 65 * h:65 * h + 65],
                                             start=True, stop=only)
                            if ch > 0:
                                nc.tensor.matmul(po[:, 65 * h:65 * h + 65],
                                                 q_fm[64 * h:64 * h + 64, ts(ch, C)],
                                                 Sb[64 * h:64 * h + 64, ch - 1, :],
                                                 start=False, stop=True)
                        # denominators at cols 0 and 65 (ones-first layout)
                        dr = drp.tile([C, 2], f32, tag="dr", name=f"dr{ch}")
                        nc.vector.reciprocal(
                            dr, po.rearrange("p (two c) -> p two c", two=2)[:, :, 0])
                        nc.vector.tensor_scalar_mul(ao[:, ch, 0:64], po[:, 1:65],
                                                    dr[:, 0:1])
                        nc.scalar.activation(ao[:, ch, 64:128], po[:, 66:130],
                                             AF.Copy, scale=dr[:, 1:2])
                        eng = nc.gpsimd if ch % 2 == 0 else nc.sync
                        eng.dma_start(cc_in[ch, :, :], ao[:, ch, :])

                with (
                    tc.tile_pool(name="psP", bufs=3, space="PSUM") as psP,
                    tc.tile_pool(name="psX", bufs=2, space="PSUM") as psX,
                ):
                    proj_mm(0, psP)
                    proj_mm(1, psP)
                    proj_rest(0, psX)
                    proj_rest(1, psX)
                    kv_chunks()
                    po_chunks()

                # wo arrives during the attention phase, well before oproj
                nc.sync.dma_start(wo_sb[:, :, :], cbf_r[:, :, 384:896])

                nc.gpsimd.collective_compute(
                    "AllToAll", mybir.AluOpType.bypass,
                    replica_groups=[list(range(8))],
                    ins=[cc_in.opt()], outs=[cc_out.opt()],
                )

                # gather (token-major), then PE transposes;
                # src s in 0-3 = batch-0 heads 2s,2s+1; s in 4-7 = batch-1
                for s in range(8):
                    eng = nc.sync if s % 2 == 0 else nc.scalar
                    eng.dma_start(gr_sb[:, s, :], cc_out[s, :, :])
                # o-projection: out.T [512, 256] (cols 0:128 batch 0, 128:256 b1)
                with (
                    tc.tile_pool(name="psO", bufs=2, space="PSUM") as psO,
                    tc.tile_pool(name="psG", bufs=3, space="PSUM") as psG,
                ):
                    for s in range(8):
                        pg = psG.tile([C, C], bf16, tag="g", name=f"pg{s}")
                        nc.tensor.transpose(pg, gr_sb[:, s, :], iden_sb[:])
                        if s % 2 == 0:
                            nc.vector.tensor_copy(g_sb[:, s, :], pg)
                        else:
                            nc.scalar.copy(g_sb[:, s, :], pg)
                    for e in range(4):
                        pf = psO.tile([128, TS], f32, tag="pf", name=f"pf{e}")
                        for bh in range(2):
                            for s in range(NF):
                                nc.tensor.matmul(pf[:, ts(bh, C)],
                                                 wo_sb[:, s, 128 * e:128 * (e + 1)],
                                                 g_sb[:, 4 * bh + s, :],
                                                 start=(s == 0), stop=(s == NF - 1))
                        osl = fin.tile([128, TS], f32, tag="osl", name=f"osl{e}")
                        nc.scalar.activation(osl, pf, AF.Identity,
                                             bias=bo_sb[:, e:e + 1])
                        eng = nc.sync if e % 2 == 0 else nc.scalar
                        eng.dma_start(out[128 * e:128 * (e + 1), :], osl)

    nc.compile()
    _CACHE["nc"] = nc
    return nc


def _in_maps(x, Wq, bq, Wk, bk, Wv, bv, Wo, bo):
    import ml_dtypes
    bf = ml_dtypes.bfloat16
    maps = []
    woT = np.ascontiguousarray(Wo.T).astype(bf)  # [512 in, 512 outIdx]
    mask = np.triu(np.ones((C, C), np.float32))
    for c in range(8):
        b, q = c // 4, c % 4
        h0, h1 = 2 * q, 2 * q + 1
        s0, s1 = slice(HD * h0, HD * (h0 + 1)), slice(HD * h1, HD * (h1 + 1))
        x2 = np.ascontiguousarray(x[b].T).astype(bf)                 # [512, 1024]
        wqk = np.concatenate([Wq[s0], Wq[s1], Wk[s0], Wk[s1]], 0).T  # [512, 256]
        wv = np.concatenate([Wv[s0], Wv[s1]], 0).T                   # [512, 128]
        cb = np.zeros((128, CBW), dtype=bf)
        for f in range(NF):
            rows = slice(128 * f, 128 * (f + 1))
            cb[:, 896 * f + 0:896 * f + 256] = wqk[rows].astype(bf)
            cb[:, 896 * f + 256:896 * f + 384] = wv[rows].astype(bf)
            cb[:, 896 * f + 384:896 * f + 896] = woT[rows]
        cf = np.zeros((128, 263), dtype=np.float32)
        cf[:, 0] = np.concatenate([bq[s0], bq[s1]])
        cf[:, 1] = np.concatenate([bk[s0], bk[s1]])
        cf[:, 2] = np.concatenate([bv[s0], bv[s1]])
        cf[:, 3:7] = bo.reshape(4, 128).T
        cf[:, 7:135] = mask
        cf[:, 135:263] = mask
        maps.append(dict(
            xT=x2,
            cbf=np.ascontiguousarray(cb),
            cf32=np.ascontiguousarray(cf),
        ))
    return maps


def kernel(x, Wq, bq, Wk, bk, Wv, bv, Wo, bo):
    from concourse import bass_utils

    nc = _build()
    maps = _in_maps(np.asarray(x), np.asarray(Wq), np.asarray(bq),
                    np.asarray(Wk), np.asarray(bk), np.asarray(Wv),
                    np.asarray(bv), np.asarray(Wo), np.asarray(bo))
    res = bass_utils.run_bass_kernel_spmd(nc, maps, core_ids=list(range(8)))
    o = np.zeros((B, T, D), np.float32)
    for c in range(8):
        s = res.results[c]["out"]                 # [512, 256] o.T slice
        o[0, C * c:C * (c + 1), :] = s[:, 0:C].T
        o[1, C * c:C * (c + 1), :] = s[:, C:2 * C].T
    return np.ascontiguousarray(o).astype(np.float32)
